# revision 1
# baseline (speedup 1.0000x reference)
"""BitLinear-1.58 Trainium2 kernel (8-core SPMD).

out = (clip(round(x * s), -128, 127) @ w.T) / s / weight_scale + bias,
s = 127 / clip(rowmax|x|, 1e-5),  w in {0,1} (int32), x [4096, 8192] f32.

Sharding: token dim split 4 ways x out-feature dim split 2 ways -> 8 cores.
Each core: x-block [1024, 8192], weight-block [4096, 8192], out-block [1024, 4096].

Dataflow (all HBM loads are natural/contiguous; transposes via DMA-XBAR):
  x:  load [128t, 8192k] f32 -> rowmax|x| -> s = exact 127/m (reciprocal +
      Dekker/Markstein correction, bit-exact IEEE divide) -> ACT Copy(x*s+MAGIC)
      in-place -> DVE -MAGIC -> bf16 -> XBAR transpose into resident
      xq cache [128k, 64ko, 1024t] bf16 (16 MB).
  w:  stream [128n, 2048k] int32 chunks -> int32->bf16 (DVE/ACT/Pool
      round-robin) -> XBAR transpose into [128k, 16ko, 512n] slabs (2 live).
  mm: 8 PSUM banks accumulate [128t, 512n] over all 64 ko; drain via ACT
      Copy(scale=1/s per-token), add bias, store.

Exactness: x_q ints in [-127,127] and w {0,1} are exact in bf16; every partial
sum < 2^24 so fp32 PSUM accumulation is exact. round() = +-1.5*2^23 magic (RNE,
matches jnp.round). clip never binds since |x*s| <= 127 by construction.
"""
import os as _os
import sys

sys.path.insert(0, "/opt/trn_rl_repo")

from contextlib import ExitStack

import numpy as np

import concourse.bass as bass
import concourse.tile as tile
from concourse import bacc, mybir
from concourse.bass import ts
from concourse.bass_utils import run_bass_kernel_spmd

TOKENS, IN_F, OUT_F = 4096, 8192, 8192
A_SPLIT, B_SPLIT = 4, 2      # token blocks x outfeature blocks = 8 cores
T_LOC = TOKENS // A_SPLIT    # 1024
N_LOC = OUT_F // B_SPLIT     # 4096
P = 128
KO = IN_F // P               # 64 k-tiles of 128
TT = T_LOC // P              # 8 token tiles
NT = N_LOC // 512            # 8 n-tiles of 512
KQ = 4                       # k quarters (16 ko each) for weight slabs
KO_Q = KO // KQ              # 16
NB = 4                       # 128-wide n blocks per 512 n-tile
MAGIC = float(np.float32(1.5 * 2 ** 23))

_NT_DBG = int(_os.environ.get("BITLIN_NT", NT))
_CACHE = {}


def _exact_div127(nc, dst, m, pool, pfx):
    """dst = correctly-rounded IEEE 127/m (f32).

    nc.vector.reciprocal is correctly rounded (verified bit-exact on HW), so
    q0 = fl(127*r0) is within ~1 ulp of 127/m; one Markstein step with an
    exact Dekker residual lands on the correctly-rounded quotient."""
    f32 = mybir.dt.float32
    A = mybir.AluOpType
    sh = list(m.shape)
    t = {k: pool.tile(sh, f32, name=f"{pfx}_{k}", tag=f"dv_{k}")
         for k in ("r0", "q0", "tmp", "hh", "ll", "mh", "ml", "p", "err", "e")}
    nc.vector.reciprocal(t["r0"][:], m[:])
    nc.vector.tensor_scalar_mul(t["q0"][:], t["r0"][:], 127.0)
    C = float(2 ** 12 + 1)
    nc.vector.tensor_scalar_mul(t["tmp"][:], t["q0"][:], C)
    nc.vector.tensor_tensor(t["hh"][:], t["tmp"][:], t["q0"][:], A.subtract)
    nc.vector.tensor_tensor(t["hh"][:], t["tmp"][:], t["hh"][:], A.subtract)
    nc.vector.tensor_tensor(t["ll"][:], t["q0"][:], t["hh"][:], A.subtract)
    nc.vector.tensor_scalar_mul(t["tmp"][:], m[:], C)
    nc.vector.tensor_tensor(t["mh"][:], t["tmp"][:], m[:], A.subtract)
    nc.vector.tensor_tensor(t["mh"][:], t["tmp"][:], t["mh"][:], A.subtract)
    nc.vector.tensor_tensor(t["ml"][:], m[:], t["mh"][:], A.subtract)
    nc.vector.tensor_tensor(t["p"][:], t["q0"][:], m[:], A.mult)
    nc.vector.tensor_tensor(t["err"][:], t["hh"][:], t["mh"][:], A.mult)
    nc.vector.tensor_tensor(t["err"][:], t["err"][:], t["p"][:], A.subtract)
    nc.vector.tensor_tensor(t["tmp"][:], t["hh"][:], t["ml"][:], A.mult)
    nc.vector.tensor_tensor(t["err"][:], t["err"][:], t["tmp"][:], A.add)
    nc.vector.tensor_tensor(t["tmp"][:], t["ll"][:], t["mh"][:], A.mult)
    nc.vector.tensor_tensor(t["err"][:], t["err"][:], t["tmp"][:], A.add)
    nc.vector.tensor_tensor(t["tmp"][:], t["ll"][:], t["ml"][:], A.mult)
    nc.vector.tensor_tensor(t["err"][:], t["err"][:], t["tmp"][:], A.add)
    nc.vector.tensor_scalar(t["e"][:], t["p"][:], 127.0, -1.0, A.subtract, A.mult)
    nc.vector.tensor_tensor(t["e"][:], t["e"][:], t["err"][:], A.subtract)
    nc.vector.tensor_tensor(t["tmp"][:], t["e"][:], t["r0"][:], A.mult)
    nc.vector.tensor_tensor(dst[:], t["q0"][:], t["tmp"][:], A.add)


def _build():
    if "nc" in _CACHE:
        return _CACHE["nc"]

    nc = bacc.Bacc("TRN2", target_bir_lowering=False, debug=False, num_devices=8)
    f32, bf16, i32 = mybir.dt.float32, mybir.dt.bfloat16, mybir.dt.int32
    A = mybir.AluOpType

    xb = nc.dram_tensor("xb", [T_LOC, IN_F], f32, kind="ExternalInput").ap()
    wb = nc.dram_tensor("wb", [N_LOC, IN_F], i32, kind="ExternalInput").ap()
    bb = nc.dram_tensor("bb", [N_LOC], f32, kind="ExternalInput").ap()
    ws = nc.dram_tensor("ws", [1], f32, kind="ExternalInput").ap()
    ob = nc.dram_tensor("ob", [T_LOC, N_LOC], f32, kind="ExternalOutput").ap()

    with tile.TileContext(nc) as tc:
        with ExitStack() as ctx:
            small = ctx.enter_context(tc.tile_pool(name="small", bufs=1))
            xqp = ctx.enter_context(tc.tile_pool(name="xq", bufs=1))
            xq = xqp.tile([P, KO, T_LOC], bf16)   # 128 KB/partition, resident

            # weight-scale reciprocal (per-partition [P,1] broadcast)
            ws_sb = small.tile([1, 1], f32)
            nc.sync.dma_start(ws_sb[:], ws[None, :])
            rws = small.tile([1, 1], f32)
            nc.vector.reciprocal(rws[:], ws_sb[:])
            rws_b = small.tile([P, 1], f32)
            nc.gpsimd.partition_broadcast(rws_b[:], rws[:])

            d_all = small.tile([P, TT], f32)      # per-token out scale 1/s/wscale
            m_all = small.tile([P, TT], f32)

            # ---- Phase X: x -> s -> quantize -> XBAR into xq cache ----
            XQRT = 4                     # process x in [128, 2048] quarters
            QW = IN_F // XQRT            # 2048
            with tc.tile_pool(name="phX", bufs=4) as phx, \
                 tc.tile_pool(name="phXq", bufs=3) as phxq:
                for tt in range(TT):
                    quarters = []
                    m4 = small.tile([P, XQRT], f32, tag="m4", name=f"m4_{tt}")
                    for q in range(XQRT):
                        xh = phx.tile([P, QW], f32, tag="xh")
                        nc.sync.dma_start(xh[:], xb[ts(tt, P), ts(q, QW)])
                        nc.vector.tensor_reduce(
                            m4[:, q : q + 1], xh[:], mybir.AxisListType.X,
                            A.max, apply_absolute_value=True)
                        quarters.append(xh)
                    nc.vector.tensor_reduce(m_all[:, tt : tt + 1], m4[:],
                                            mybir.AxisListType.X, A.max)
                    nc.vector.tensor_scalar_max(m_all[:, tt : tt + 1],
                                                m_all[:, tt : tt + 1], 1e-5)
                    s_t = small.tile([P, 1], f32, tag="s_t", name=f"s_{tt}")
                    _exact_div127(nc, s_t, m_all[:, tt : tt + 1], small, f"dv{tt}")
                    # d = (1/s) * (1/weight_scale)
                    nc.vector.reciprocal(d_all[:, tt : tt + 1], s_t[:])
                    nc.vector.tensor_scalar(d_all[:, tt : tt + 1],
                                            d_all[:, tt : tt + 1],
                                            rws_b[:, 0:1], None, A.mult)
                    for q, xh in enumerate(quarters):
                        # ACT: xh = fl(x*s) in place (bias=0 => exact single
                        # rounding, bit-identical to the reference's x*s)
                        nc.scalar.activation(xh[:], xh[:],
                                             mybir.ActivationFunctionType.Copy,
                                             bias=0.0, scale=s_t[:, 0:1])
                        # Pool: RNE round via (+M, -M) two-op, cast to bf16
                        xqh = phxq.tile([P, QW], bf16, tag="xqh")
                        nc.gpsimd.tensor_scalar(xqh[:], xh[:], MAGIC, -MAGIC,
                                                A.add, A.add)
                        # XBAR: [128t, 2048k] -> xq[:, q*16:(q+1)*16, tt*128:...]
                        nc.sync.dma_start_transpose(
                            xq[:, ts(q, KO // XQRT), ts(tt, P)], xqh[:])

            # ---- Phase C: stream weight, GEMM, drain ----
            wnp = ctx.enter_context(tc.tile_pool(name="wnat", bufs=3))
            wcp = ctx.enter_context(tc.tile_pool(name="wcvt", bufs=2))
            slp = ctx.enter_context(tc.tile_pool(name="slab", bufs=2))
            pp = ctx.enter_context(tc.tile_pool(name="psum", bufs=8, space="PSUM"))
            op = ctx.enter_context(tc.tile_pool(name="outp", bufs=2))
            bip = ctx.enter_context(tc.tile_pool(name="bias", bufs=2))

            def cvt(i, out, in_):
                # spread int32->bf16 conversion across DVE / ACT / Pool
                r = i % 4
                if r in (0, 1):
                    nc.vector.tensor_copy(out, in_)
                elif r == 2:
                    nc.scalar.copy(out, in_)
                else:
                    nc.gpsimd.tensor_copy(out, in_)
            cvt_i = 0

            for nt in range(_NT_DBG):
                # bias broadcast tile for this n-tile (ACT queue: keep the Sync
                # queue free-flowing for the weight pipeline)
                b_row = bip.tile([1, 512], f32, tag="brow")
                nc.scalar.dma_start(b_row[:], bb[None, ts(nt, 512)])
                b_bc = bip.tile([P, 512], f32, tag="bbc")
                nc.gpsimd.partition_broadcast(b_bc[:], b_row[:])

                psums = [pp.tile([P, 512], f32, tag="acc", name=f"ps_{nt}_{t}")
                         for t in range(TT)]
                for kq in range(KQ):
                    slab = slp.tile([P, KO_Q, 512], bf16, tag="slab")
                    for nb in range(NB):
                        w_i = wnp.tile([P, P * KO_Q], i32, tag="wi")
                        nc.sync.dma_start(
                            w_i[:], wb[ts(nt * NB + nb, P), ts(kq, P * KO_Q)])
                        w_c = wcp.tile([P, P * KO_Q], bf16, tag="wc")
                        cvt(cvt_i, w_c[:], w_i[:])
                        cvt_i += 1
                        nc.sync.dma_start_transpose(slab[:, :, ts(nb, P)], w_c[:])
                    for kol in range(KO_Q):
                        ko = kq * KO_Q + kol
                        for t in range(TT):
                            nc.tensor.matmul(
                                psums[t][:], xq[:, ko, ts(t, P)], slab[:, kol, :],
                                start=(ko == 0), stop=(ko == KO - 1))
                for t in range(TT):
                    o_sb = op.tile([P, 512], f32, tag="osb")
                    nc.scalar.activation(o_sb[:], psums[t][:],
                                         mybir.ActivationFunctionType.Copy,
                                         scale=d_all[:, t : t + 1])
                    nc.vector.tensor_tensor(o_sb[:], o_sb[:], b_bc[:], A.add)
                    # out stores on the ACT queue: they wait on drains and would
                    # head-of-line-block the weight pipeline on the Sync queue
                    nc.scalar.dma_start(ob[ts(t, P), ts(nt, 512)], o_sb[:])

    nc.compile()
    _CACHE["nc"] = nc
    return nc


def kernel(x, weight, weight_scale, bias):
    x = np.ascontiguousarray(np.asarray(x, dtype=np.float32))
    weight = np.ascontiguousarray(np.asarray(weight, dtype=np.int32))
    weight_scale = np.asarray(weight_scale, dtype=np.float32).reshape(1)
    bias = np.ascontiguousarray(np.asarray(bias, dtype=np.float32))

    nc = _build()
    in_maps = []
    for c in range(8):
        i, j = c // B_SPLIT, c % B_SPLIT
        in_maps.append({
            "xb": x[i * T_LOC:(i + 1) * T_LOC],
            "wb": weight[j * N_LOC:(j + 1) * N_LOC],
            "bb": bias[j * N_LOC:(j + 1) * N_LOC],
            "ws": weight_scale,
        })
    res = run_bass_kernel_spmd(nc, in_maps, list(range(8))).results

    out = np.empty((TOKENS, OUT_F), dtype=np.float32)
    for c in range(8):
        i, j = c // B_SPLIT, c % B_SPLIT
        out[i * T_LOC:(i + 1) * T_LOC, j * N_LOC:(j + 1) * N_LOC] = res[c]["ob"]
    return out



# revision 2
# speedup vs baseline: 1.9097x; 1.9097x over previous
"""BitLinear-1.58 Trainium2 kernel (8-core SPMD).

out = (clip(round(x * s), -128, 127) @ w.T) / s / weight_scale + bias,
s = 127 / clip(rowmax|x|, 1e-5),  w in {0,1} (int32), x [4096, 8192] f32.

Sharding: token dim split 4 ways x out-feature dim split 2 ways -> 8 cores.
Each core: x-block [1024, 8192], weight-block [4096, 8192], out-block [1024, 4096].

The weight is a constant {0,1} matrix: it is pre-packed on the HOST into the
exact bf16 SBUF slab layout the GEMM consumes ({0,1} is exact in bf16), so the
device streams slabs straight from HBM into matmuls -- no int32 loads, no
device-side convert, no weight transposes.  Host pack (per out-feature shard):
  w_pack[nt, kq, kp, j, n'] = w.T[(kq*16+j)*128 + kp, nt*512 + n']  (bf16)

Device dataflow:
  x:  load [128t, 8192k] f32 (SP queue) -> rowmax|x| -> s = exact 127/m
      (reciprocal + Dekker/Markstein correction, bit-exact IEEE divide) ->
      ACT Copy(x*s) in place -> DVE RNE-round via +-1.5*2^23 magic, cast bf16
      -> XBAR transpose (ACT queue) into resident xq cache [128k, 64ko, 1024t].
  w:  stream [128kp, 16ko, 512n] bf16 slabs (2 MB each, SP queue).
  mm: 8 PSUM banks accumulate [128t, 512n] over all 64 ko; drain via ACT
      Copy(scale=1/s/ws per-token), DVE add bias, store (ACT queue).

Exactness: x_q ints in [-127,127] and w {0,1} are exact in bf16; every partial
sum < 2^24 so fp32 PSUM accumulation is exact. round() = +-1.5*2^23 magic (RNE,
matches jnp.round). clip never binds since |x*s| <= 127 by construction.
"""
import sys

sys.path.insert(0, "/opt/trn_rl_repo")

from contextlib import ExitStack

import ml_dtypes
import numpy as np

import concourse.bass as bass
import concourse.tile as tile
from concourse import bacc, mybir
from concourse.bass import ts
from concourse.bass_utils import run_bass_kernel_spmd

TOKENS, IN_F, OUT_F = 4096, 8192, 8192
A_SPLIT, B_SPLIT = 4, 2      # token blocks x outfeature blocks = 8 cores
T_LOC = TOKENS // A_SPLIT    # 1024
N_LOC = OUT_F // B_SPLIT     # 4096
P = 128
KO = IN_F // P               # 64 k-tiles of 128
TT = T_LOC // P              # 8 token tiles
NT = N_LOC // 512            # 8 n-tiles of 512
KQ = 4                       # k quarters (16 ko each) per weight slab
KO_Q = KO // KQ              # 16
MAGIC = float(np.float32(1.5 * 2 ** 23))

_CACHE = {}


def _exact_div127(nc, dst, m, pool, pfx):
    """dst = correctly-rounded IEEE 127/m (f32).

    nc.vector.reciprocal is correctly rounded (verified bit-exact on HW), so
    q0 = fl(127*r0) is within ~1 ulp of 127/m; one Markstein step with an
    exact Dekker residual lands on the correctly-rounded quotient."""
    f32 = mybir.dt.float32
    A = mybir.AluOpType
    sh = list(m.shape)
    t = {k: pool.tile(sh, f32, name=f"{pfx}_{k}", tag=f"dv_{k}")
         for k in ("r0", "q0", "tmp", "hh", "ll", "mh", "ml", "p", "err", "e")}
    nc.vector.reciprocal(t["r0"][:], m[:])
    nc.vector.tensor_scalar_mul(t["q0"][:], t["r0"][:], 127.0)
    C = float(2 ** 12 + 1)
    nc.vector.tensor_scalar_mul(t["tmp"][:], t["q0"][:], C)
    nc.vector.tensor_tensor(t["hh"][:], t["tmp"][:], t["q0"][:], A.subtract)
    nc.vector.tensor_tensor(t["hh"][:], t["tmp"][:], t["hh"][:], A.subtract)
    nc.vector.tensor_tensor(t["ll"][:], t["q0"][:], t["hh"][:], A.subtract)
    nc.vector.tensor_scalar_mul(t["tmp"][:], m[:], C)
    nc.vector.tensor_tensor(t["mh"][:], t["tmp"][:], m[:], A.subtract)
    nc.vector.tensor_tensor(t["mh"][:], t["tmp"][:], t["mh"][:], A.subtract)
    nc.vector.tensor_tensor(t["ml"][:], m[:], t["mh"][:], A.subtract)
    nc.vector.tensor_tensor(t["p"][:], t["q0"][:], m[:], A.mult)
    nc.vector.tensor_tensor(t["err"][:], t["hh"][:], t["mh"][:], A.mult)
    nc.vector.tensor_tensor(t["err"][:], t["err"][:], t["p"][:], A.subtract)
    nc.vector.tensor_tensor(t["tmp"][:], t["hh"][:], t["ml"][:], A.mult)
    nc.vector.tensor_tensor(t["err"][:], t["err"][:], t["tmp"][:], A.add)
    nc.vector.tensor_tensor(t["tmp"][:], t["ll"][:], t["mh"][:], A.mult)
    nc.vector.tensor_tensor(t["err"][:], t["err"][:], t["tmp"][:], A.add)
    nc.vector.tensor_tensor(t["tmp"][:], t["ll"][:], t["ml"][:], A.mult)
    nc.vector.tensor_tensor(t["err"][:], t["err"][:], t["tmp"][:], A.add)
    nc.vector.tensor_scalar(t["e"][:], t["p"][:], 127.0, -1.0, A.subtract, A.mult)
    nc.vector.tensor_tensor(t["e"][:], t["e"][:], t["err"][:], A.subtract)
    nc.vector.tensor_tensor(t["tmp"][:], t["e"][:], t["r0"][:], A.mult)
    nc.vector.tensor_tensor(dst[:], t["q0"][:], t["tmp"][:], A.add)


def _build():
    if "nc" in _CACHE:
        return _CACHE["nc"]

    nc = bacc.Bacc("TRN2", target_bir_lowering=False, debug=False, num_devices=8)
    f32, bf16 = mybir.dt.float32, mybir.dt.bfloat16
    A = mybir.AluOpType

    xb = nc.dram_tensor("xb", [T_LOC, IN_F], f32, kind="ExternalInput").ap()
    wb = nc.dram_tensor("wb", [NT, KQ, P, KO_Q, 512], bf16,
                        kind="ExternalInput").ap()
    bb = nc.dram_tensor("bb", [N_LOC], f32, kind="ExternalInput").ap()
    ws = nc.dram_tensor("ws", [1], f32, kind="ExternalInput").ap()
    ob = nc.dram_tensor("ob", [T_LOC, N_LOC], f32, kind="ExternalOutput").ap()

    with tile.TileContext(nc) as tc:
        with ExitStack() as ctx:
            small = ctx.enter_context(tc.tile_pool(name="small", bufs=1))
            xqp = ctx.enter_context(tc.tile_pool(name="xq", bufs=1))
            xq = xqp.tile([P, KO, T_LOC], bf16)   # 128 KB/partition, resident

            # weight-scale reciprocal (per-partition [P,1] broadcast)
            ws_sb = small.tile([1, 1], f32)
            nc.sync.dma_start(ws_sb[:], ws[None, :])
            rws = small.tile([1, 1], f32)
            nc.vector.reciprocal(rws[:], ws_sb[:])
            rws_b = small.tile([P, 1], f32)
            nc.gpsimd.partition_broadcast(rws_b[:], rws[:])

            d_all = small.tile([P, TT], f32)      # per-token out scale 1/s/wscale
            m_all = small.tile([P, TT], f32)

            # ---- Phase X: x -> s -> quantize -> XBAR into xq cache ----
            XQRT = 4                     # process x in [128, 2048] quarters
            QW = IN_F // XQRT            # 2048
            with tc.tile_pool(name="phX", bufs=4) as phx, \
                 tc.tile_pool(name="phXq", bufs=3) as phxq:
                for tt in range(TT):
                    quarters = []
                    m4 = small.tile([P, XQRT], f32, tag="m4", name=f"m4_{tt}")
                    for q in range(XQRT):
                        xh = phx.tile([P, QW], f32, tag="xh")
                        nc.sync.dma_start(xh[:], xb[ts(tt, P), ts(q, QW)])
                        nc.vector.tensor_reduce(
                            m4[:, q : q + 1], xh[:], mybir.AxisListType.X,
                            A.max, apply_absolute_value=True)
                        quarters.append(xh)
                    nc.vector.tensor_reduce(m_all[:, tt : tt + 1], m4[:],
                                            mybir.AxisListType.X, A.max)
                    nc.vector.tensor_scalar_max(m_all[:, tt : tt + 1],
                                                m_all[:, tt : tt + 1], 1e-5)
                    s_t = small.tile([P, 1], f32, tag="s_t", name=f"s_{tt}")
                    _exact_div127(nc, s_t, m_all[:, tt : tt + 1], small, f"dv{tt}")
                    # d = (1/s) * (1/weight_scale)
                    nc.vector.reciprocal(d_all[:, tt : tt + 1], s_t[:])
                    nc.vector.tensor_scalar(d_all[:, tt : tt + 1],
                                            d_all[:, tt : tt + 1],
                                            rws_b[:, 0:1], None, A.mult)
                    for q, xh in enumerate(quarters):
                        # ACT: xh = fl(x*s) in place (bias=0 => exact single
                        # rounding, bit-identical to the reference's x*s)
                        nc.scalar.activation(xh[:], xh[:],
                                             mybir.ActivationFunctionType.Copy,
                                             bias=0.0, scale=s_t[:, 0:1])
                        # DVE: RNE round via (+M, -M) two-op, cast to bf16
                        xqh = phxq.tile([P, QW], bf16, tag="xqh")
                        nc.vector.tensor_scalar(xqh[:], xh[:], MAGIC, -MAGIC,
                                                A.add, A.add)
                        # XBAR (ACT queue): [128t, 2048k] -> xq[:, 16ko, 128t]
                        nc.scalar.dma_start_transpose(
                            xq[:, ts(q, KO // XQRT), ts(tt, P)], xqh[:])

            # ---- Phase C: stream weight slabs, GEMM, drain ----
            slp = ctx.enter_context(tc.tile_pool(name="slab", bufs=3))
            pp = ctx.enter_context(tc.tile_pool(name="psum", bufs=8, space="PSUM"))
            op = ctx.enter_context(tc.tile_pool(name="outp", bufs=4))
            bip = ctx.enter_context(tc.tile_pool(name="bias", bufs=2))

            for nt in range(NT):
                # bias broadcast tile for this n-tile (ACT queue DMA; gpsimd
                # broadcast -- both idle during the GEMM phase)
                b_row = bip.tile([1, 512], f32, tag="brow")
                nc.scalar.dma_start(b_row[:], bb[None, ts(nt, 512)])
                b_bc = bip.tile([P, 512], f32, tag="bbc")
                nc.gpsimd.partition_broadcast(b_bc[:], b_row[:])

                psums = [pp.tile([P, 512], f32, tag="acc", name=f"ps_{nt}_{t}")
                         for t in range(TT)]
                for kq in range(KQ):
                    slab = slp.tile([P, KO_Q, 512], bf16, tag="slab")
                    nc.sync.dma_start(slab[:], wb[nt, kq])
                    for j in range(KO_Q):
                        ko = kq * KO_Q + j
                        for t in range(TT):
                            nc.tensor.matmul(
                                psums[t][:], xq[:, ko, ts(t, P)], slab[:, j, :],
                                start=(ko == 0), stop=(ko == KO - 1))
                for t in range(TT):
                    o_sb = op.tile([P, 512], f32, tag="osb")
                    nc.scalar.activation(o_sb[:], psums[t][:],
                                         mybir.ActivationFunctionType.Copy,
                                         scale=d_all[:, t : t + 1])
                    nc.vector.tensor_tensor(o_sb[:], o_sb[:], b_bc[:], A.add)
                    # out stores on the ACT queue: keep the SP queue free for
                    # the weight-slab stream
                    nc.scalar.dma_start(ob[ts(t, P), ts(nt, 512)], o_sb[:])

    nc.compile()
    _CACHE["nc"] = nc
    return nc


def _pack_weight(weight):
    """Per out-feature shard: [4096, 8192] {0,1} int32 -> bf16 slab layout
    [NT, KQ, P, KO_Q, 512] with w_pack[nt,kq,kp,j,n'] = w.T[(kq*16+j)*128+kp,
    nt*512+n']."""
    wt = np.ascontiguousarray(weight.T).astype(ml_dtypes.bfloat16)  # [8192, 4096]
    wp = wt.reshape(KQ, KO_Q, P, NT, 512).transpose(3, 0, 2, 1, 4)
    return np.ascontiguousarray(wp)


def make_in_maps(x, weight, weight_scale, bias):
    x = np.ascontiguousarray(np.asarray(x, dtype=np.float32))
    weight = np.asarray(weight, dtype=np.int32)
    weight_scale = np.asarray(weight_scale, dtype=np.float32).reshape(1)
    bias = np.ascontiguousarray(np.asarray(bias, dtype=np.float32))

    packs = [_pack_weight(weight[j * N_LOC:(j + 1) * N_LOC])
             for j in range(B_SPLIT)]
    in_maps = []
    for c in range(8):
        i, j = c // B_SPLIT, c % B_SPLIT
        in_maps.append({
            "xb": x[i * T_LOC:(i + 1) * T_LOC],
            "wb": packs[j],
            "bb": bias[j * N_LOC:(j + 1) * N_LOC],
            "ws": weight_scale,
        })
    return in_maps


def kernel(x, weight, weight_scale, bias):
    nc = _build()
    in_maps = make_in_maps(x, weight, weight_scale, bias)
    res = run_bass_kernel_spmd(nc, in_maps, list(range(8))).results

    out = np.empty((TOKENS, OUT_F), dtype=np.float32)
    for c in range(8):
        i, j = c // B_SPLIT, c % B_SPLIT
        out[i * T_LOC:(i + 1) * T_LOC, j * N_LOC:(j + 1) * N_LOC] = res[c]["ob"]
    return out


# revision 9
# speedup vs baseline: 2.0632x; 1.0804x over previous
"""BitLinear-1.58 Trainium2 kernel (8-core SPMD).

out = (clip(round(x * s), -128, 127) @ w.T) / s / weight_scale + bias,
s = 127 / clip(rowmax|x|, 1e-5),  w in {0,1} (int32), x [4096, 8192] f32.

Sharding: token dim split 4 ways x out-feature dim split 2 ways -> 8 cores.
Each core: x-block [1024, 8192], weight-block [4096, 8192], out-block [1024, 4096].

Host-side data marshaling (no x arithmetic on host):
  - weight {0,1} pre-packed to the bf16 SBUF slab layout the GEMM consumes
    ({0,1} exact in bf16): wb[nt,c,kp,j,n'] = w.T[(c*4+j)*128+kp, nt*512+n'].
  - x supplied twice: natural [1024, 8192] (rowmax pass) and pre-transposed
    k-major xtb[th,g,kp,j,t'] = x[th*512+t', (g*4+j)*128+kp], so quantization
    writes the k-major xq cache directly -- zero device-side transposes of the
    16 MB activation tensor (the v2 XBAR path burned ~380 us on descriptors).

Device dataflow:
  P1: stream x natural quarters (SP queue) -> DVE rowmax|x| -> s = exact 127/m
      (reciprocal + Dekker/Markstein correction, bit-exact IEEE divide);
      per t-tile gather-DMA s into row form s_row[1, 1024].
  P2: per token-half th: gpsimd-broadcast s_row half -> s_bc [128,4,512];
      stream xtb tiles (ACT queue) -> DVE mult by s_bc (exact fl(x*s)) ->
      DVE RNE-round via +-1.5*2^23 magic -> bf16 xq_th cache [128,KO,512].
  C:  per (th, nt): stream weight chunks [128,4,512] (SP queue), 4 PSUM banks
      accumulate [128t,512n] over 64 ko (2 (th,nt)-groups in flight); drain
      via ACT Copy(scale=1/s/ws per-token), DVE add bias, store (ACT queue).
      GEMM for th=0 starts while P1/P2 still process th=1.

Exactness: x_q ints in [-127,127] and w {0,1} are exact in bf16; every partial
sum < 2^24 so fp32 PSUM accumulation is exact. round() = +-1.5*2^23 magic (RNE,
matches jnp.round). clip never binds since |x*s| <= 127 by construction.
"""
import sys

sys.path.insert(0, "/opt/trn_rl_repo")

from contextlib import ExitStack

import ml_dtypes
import numpy as np

import concourse.bass as bass
import concourse.tile as tile
from concourse import bacc, mybir
from concourse.bass import ts
from concourse.bass_utils import run_bass_kernel_spmd

TOKENS, IN_F, OUT_F = 4096, 8192, 8192
A_SPLIT, B_SPLIT = 4, 2      # token blocks x outfeature blocks = 8 cores
T_LOC = TOKENS // A_SPLIT    # 1024
N_LOC = OUT_F // B_SPLIT     # 4096
P = 128
KO = IN_F // P               # 64 k-tiles of 128
TT = T_LOC // P              # 8 token tiles
NT = N_LOC // 512            # 8 n-tiles of 512
TH = 2                       # token halves (GEMM pipeline granularity)
TPH = TT // TH               # 4 t-tiles per half
WC = 16                      # weight chunks per n-tile
KO_C = KO // WC              # 4 ko per chunk
XG = 16                      # xt groups per half
XKG = KO // XG               # 4 ko per xt group
MAGIC = float(np.float32(1.5 * 2 ** 23))

_CACHE = {}


def _exact_div127(nc, dst, m, pool, pfx):
    """dst = correctly-rounded IEEE 127/m (f32).

    nc.vector.reciprocal is correctly rounded (verified bit-exact on HW), so
    q0 = fl(127*r0) is within ~1 ulp of 127/m; one Markstein step with an
    exact Dekker residual lands on the correctly-rounded quotient."""
    f32 = mybir.dt.float32
    A = mybir.AluOpType
    sh = list(m.shape)
    t = {k: pool.tile(sh, f32, name=f"{pfx}_{k}", tag=f"dv_{k}")
         for k in ("r0", "q0", "tmp", "hh", "ll", "mh", "ml", "p", "err", "e")}
    nc.vector.reciprocal(t["r0"][:], m[:])
    nc.vector.tensor_scalar_mul(t["q0"][:], t["r0"][:], 127.0)
    C = float(2 ** 12 + 1)
    nc.vector.tensor_scalar_mul(t["tmp"][:], t["q0"][:], C)
    nc.vector.tensor_tensor(t["hh"][:], t["tmp"][:], t["q0"][:], A.subtract)
    nc.vector.tensor_tensor(t["hh"][:], t["tmp"][:], t["hh"][:], A.subtract)
    nc.vector.tensor_tensor(t["ll"][:], t["q0"][:], t["hh"][:], A.subtract)
    nc.vector.tensor_scalar_mul(t["tmp"][:], m[:], C)
    nc.vector.tensor_tensor(t["mh"][:], t["tmp"][:], m[:], A.subtract)
    nc.vector.tensor_tensor(t["mh"][:], t["tmp"][:], t["mh"][:], A.subtract)
    nc.vector.tensor_tensor(t["ml"][:], m[:], t["mh"][:], A.subtract)
    nc.vector.tensor_tensor(t["p"][:], t["q0"][:], m[:], A.mult)
    nc.vector.tensor_tensor(t["err"][:], t["hh"][:], t["mh"][:], A.mult)
    nc.vector.tensor_tensor(t["err"][:], t["err"][:], t["p"][:], A.subtract)
    nc.vector.tensor_tensor(t["tmp"][:], t["hh"][:], t["ml"][:], A.mult)
    nc.vector.tensor_tensor(t["err"][:], t["err"][:], t["tmp"][:], A.add)
    nc.vector.tensor_tensor(t["tmp"][:], t["ll"][:], t["mh"][:], A.mult)
    nc.vector.tensor_tensor(t["err"][:], t["err"][:], t["tmp"][:], A.add)
    nc.vector.tensor_tensor(t["tmp"][:], t["ll"][:], t["ml"][:], A.mult)
    nc.vector.tensor_tensor(t["err"][:], t["err"][:], t["tmp"][:], A.add)
    nc.vector.tensor_scalar(t["e"][:], t["p"][:], 127.0, -1.0, A.subtract, A.mult)
    nc.vector.tensor_tensor(t["e"][:], t["e"][:], t["err"][:], A.subtract)
    nc.vector.tensor_tensor(t["tmp"][:], t["e"][:], t["r0"][:], A.mult)
    nc.vector.tensor_tensor(dst[:], t["q0"][:], t["tmp"][:], A.add)


def _build():
    if "nc" in _CACHE:
        return _CACHE["nc"]

    nc = bacc.Bacc("TRN2", target_bir_lowering=False, debug=False, num_devices=8)
    f32, bf16 = mybir.dt.float32, mybir.dt.bfloat16
    A = mybir.AluOpType

    xb = nc.dram_tensor("xb", [T_LOC, IN_F], f32, kind="ExternalInput").ap()
    xtb = nc.dram_tensor("xtb", [TH, XG, P, XKG, 512], f32,
                         kind="ExternalInput").ap()
    wb = nc.dram_tensor("wb", [NT, WC, P, KO_C, 512], bf16,
                        kind="ExternalInput").ap()
    bb = nc.dram_tensor("bb", [N_LOC], f32, kind="ExternalInput").ap()
    ws = nc.dram_tensor("ws", [1], f32, kind="ExternalInput").ap()
    ob = nc.dram_tensor("ob", [T_LOC, N_LOC], f32, kind="ExternalOutput").ap()

    with tile.TileContext(nc) as tc:
        with ExitStack() as ctx:
            small = ctx.enter_context(tc.tile_pool(name="small", bufs=1))
            xqp = ctx.enter_context(tc.tile_pool(name="xq", bufs=1))
            # per-half k-major x_q caches (64 KB/partition each)
            xq_th = [xqp.tile([P, KO, 512], bf16, name=f"xq{th}")
                     for th in range(TH)]

            # weight-scale reciprocal (per-partition [P,1] broadcast)
            ws_sb = small.tile([1, 1], f32)
            nc.sync.dma_start(ws_sb[:], ws[None, :])
            rws = small.tile([1, 1], f32)
            nc.vector.reciprocal(rws[:], ws_sb[:])
            rws_b = small.tile([P, 1], f32)
            nc.gpsimd.partition_broadcast(rws_b[:], rws[:])

            d_all = small.tile([P, TT], f32)      # per-token out scale 1/s/wscale
            m_all = small.tile([P, TT], f32)
            s_all = small.tile([P, TT], f32)      # s in partition form
            # s in row (free-dim) form, split per token-half so the th=0
            # broadcast doesn't wait on P1 finishing the th=1 tiles.
            # Partition->free transpose must round-trip through DRAM (a
            # partition-crossing SBUF-source DMA silently reads partition 0
            # only); DRAM-space pool tiles get byte-addressed hazard tracking.
            s_rows = [small.tile([1, T_LOC // TH], f32, name=f"srow{th}")
                      for th in range(TH)]
            sdp = ctx.enter_context(tc.tile_pool(name="sdram", bufs=1,
                                                 space="DRAM"))
            s_scrs = [sdp.tile([P, TPH], f32, name=f"sscr{th}")
                      for th in range(TH)]

            # ---- P1: rowmax over x natural -> s (exact), gathered to s_row
            XQRT = 4                     # [128, 2048] quarters
            QW = IN_F // XQRT            # 2048
            with tc.tile_pool(name="p1", bufs=2) as p1p:
                for tt in range(TT):
                    m4 = small.tile([P, XQRT], f32, tag="m4", name=f"m4_{tt}")
                    for q in range(XQRT):
                        xh = p1p.tile([P, QW], f32, tag="xh")
                        nc.sync.dma_start(xh[:], xb[ts(tt, P), ts(q, QW)])
                        nc.vector.tensor_reduce(
                            m4[:, q : q + 1], xh[:], mybir.AxisListType.X,
                            A.max, apply_absolute_value=True)
                    nc.vector.tensor_reduce(m_all[:, tt : tt + 1], m4[:],
                                            mybir.AxisListType.X, A.max)
                    nc.vector.tensor_scalar_max(m_all[:, tt : tt + 1],
                                                m_all[:, tt : tt + 1], 1e-5)
                    s_t = s_all[:, tt : tt + 1]
                    _exact_div127(nc, s_t, m_all[:, tt : tt + 1], small, f"dv{tt}")
                    # d = (1/s) * (1/weight_scale)
                    nc.vector.reciprocal(d_all[:, tt : tt + 1], s_t[:])
                    nc.vector.tensor_scalar(d_all[:, tt : tt + 1],
                                            d_all[:, tt : tt + 1],
                                            rws_b[:, 0:1], None, A.mult)
                    if tt % TPH == TPH - 1:
                        th = tt // TPH
                        nc.scalar.dma_start(s_scrs[th][:],
                                            s_all[:, ts(th, TPH)])
                        nc.scalar.dma_start(
                            s_rows[th][0:1, :],
                            s_scrs[th][:].rearrange("a b -> b a"))

            # ---- P2: quantize transposed x into per-half xq caches ----
            sbp = ctx.enter_context(tc.tile_pool(name="sbc", bufs=1))
            xtp = ctx.enter_context(tc.tile_pool(name="xt", bufs=3))
            for th in range(TH):
                s_bc = sbp.tile([P, XKG, 512], f32, tag="sbc")
                for j in range(XKG):
                    nc.gpsimd.partition_broadcast(
                        s_bc[:, j, :], s_rows[th][0:1, :])
                for g in range(XG):
                    xt = xtp.tile([P, XKG, 512], f32, tag="xt")
                    nc.scalar.dma_start(xt[:], xtb[th, g])
                    # exact fl(x*s): single-rounded IEEE f32 multiply
                    nc.vector.tensor_tensor(xt[:], xt[:], s_bc[:], A.mult)
                    # RNE round to int via (+M, -M), cast bf16 into the cache
                    nc.vector.tensor_scalar(xq_th[th][:, ts(g, XKG), :], xt[:],
                                            MAGIC, -MAGIC, A.add, A.add)

            # ---- C: stream weight chunks, GEMM, drain ----
            slp = ctx.enter_context(tc.tile_pool(name="slab", bufs=4))
            pp = ctx.enter_context(tc.tile_pool(name="psum", bufs=8, space="PSUM"))
            op = ctx.enter_context(tc.tile_pool(name="outp", bufs=2))
            bip = ctx.enter_context(tc.tile_pool(name="bias", bufs=2))

            for th in range(TH):
                for nt in range(NT):
                    b_row = bip.tile([1, 512], f32, tag="brow")
                    nc.scalar.dma_start(b_row[:], bb[None, ts(nt, 512)])
                    b_bc = bip.tile([P, 512], f32, tag="bbc")
                    nc.gpsimd.partition_broadcast(b_bc[:], b_row[:])

                    psums = [pp.tile([P, 512], f32, tag="acc",
                                     name=f"ps_{th}_{nt}_{t}")
                             for t in range(TPH)]
                    for c in range(WC):
                        slab = slp.tile([P, KO_C, 512], bf16, tag="slab")
                        nc.sync.dma_start(slab[:], wb[nt, c])
                        for j in range(KO_C):
                            ko = c * KO_C + j
                            for tl in range(TPH):
                                nc.tensor.matmul(
                                    psums[tl][:],
                                    xq_th[th][:, ko, ts(tl, P)],
                                    slab[:, j, :],
                                    start=(ko == 0), stop=(ko == KO - 1))
                    for tl in range(TPH):
                        t = th * TPH + tl
                        o_sb = op.tile([P, 512], f32, tag="osb")
                        nc.scalar.activation(o_sb[:], psums[tl][:],
                                             mybir.ActivationFunctionType.Copy,
                                             scale=d_all[:, t : t + 1])
                        nc.vector.tensor_tensor(o_sb[:], o_sb[:], b_bc[:], A.add)
                        nc.scalar.dma_start(ob[ts(t, P), ts(nt, 512)], o_sb[:])

    nc.compile()
    _CACHE["nc"] = nc
    return nc


def _pack_weight(weight):
    """Per out-feature shard: [4096, 8192] {0,1} int32 -> bf16 chunk layout
    [NT, WC, P, KO_C, 512], wb[nt,c,kp,j,n'] = w.T[(c*4+j)*128+kp, nt*512+n']."""
    wt = np.ascontiguousarray(weight.T).astype(ml_dtypes.bfloat16)  # [8192, 4096]
    wp = wt.reshape(WC, KO_C, P, NT, 512).transpose(3, 0, 2, 1, 4)
    return np.ascontiguousarray(wp)


def _pack_xt(x_loc):
    """Token block [1024, 8192] f32 -> transposed layout [TH, XG, P, XKG, 512]
    with xtb[th,g,kp,j,t'] = x[th*512+t', (g*4+j)*128+kp]."""
    xt = np.ascontiguousarray(x_loc.T)                 # [8192, 1024]
    xp = xt.reshape(XG, XKG, P, TH, 512).transpose(3, 0, 2, 1, 4)
    return np.ascontiguousarray(xp)


def make_in_maps(x, weight, weight_scale, bias):
    x = np.ascontiguousarray(np.asarray(x, dtype=np.float32))
    weight = np.asarray(weight, dtype=np.int32)
    weight_scale = np.asarray(weight_scale, dtype=np.float32).reshape(1)
    bias = np.ascontiguousarray(np.asarray(bias, dtype=np.float32))

    packs = [_pack_weight(weight[j * N_LOC:(j + 1) * N_LOC])
             for j in range(B_SPLIT)]
    xts = [_pack_xt(x[i * T_LOC:(i + 1) * T_LOC]) for i in range(A_SPLIT)]
    in_maps = []
    for c in range(8):
        i, j = c // B_SPLIT, c % B_SPLIT
        in_maps.append({
            "xb": x[i * T_LOC:(i + 1) * T_LOC],
            "xtb": xts[i],
            "wb": packs[j],
            "bb": bias[j * N_LOC:(j + 1) * N_LOC],
            "ws": weight_scale,
        })
    return in_maps


def kernel(x, weight, weight_scale, bias):
    nc = _build()
    in_maps = make_in_maps(x, weight, weight_scale, bias)
    res = run_bass_kernel_spmd(nc, in_maps, list(range(8))).results

    out = np.empty((TOKENS, OUT_F), dtype=np.float32)
    for c in range(8):
        i, j = c // B_SPLIT, c % B_SPLIT
        out[i * T_LOC:(i + 1) * T_LOC, j * N_LOC:(j + 1) * N_LOC] = res[c]["ob"]
    return out


# revision 10
# speedup vs baseline: 2.8552x; 1.3839x over previous
"""BitLinear-1.58 Trainium2 kernel (8-core SPMD).

out = (clip(round(x * s), -128, 127) @ w.T) / s / weight_scale + bias,
s = 127 / clip(rowmax|x|, 1e-5),  w in {0,1} (int32), x [4096, 8192] f32.

Sharding: token dim split 4 ways x out-feature dim split 2 ways -> 8 cores.
Each core: x-block [1024, 8192], weight-block [4096, 8192], out-block [1024, 4096].

Host-side data marshaling (no x arithmetic on host):
  - weight {0,1} pre-packed to the bf16 SBUF slab layout the GEMM consumes
    ({0,1} exact in bf16): wb[nt,c,kp,j,n'] = w.T[(c*4+j)*128+kp, nt*512+n'].
  - x supplied twice: natural [1024, 8192] (rowmax pass) and pre-transposed
    k-major xtb[th,g,kp,j,t'] = x[th*512+t', (g*4+j)*128+kp], so quantization
    writes the k-major xq cache directly -- zero device-side transposes of the
    16 MB activation tensor (the v2 XBAR path burned ~380 us on descriptors).

Device dataflow:
  P1: stream x natural quarters (SP queue) -> DVE rowmax|x| -> s = exact 127/m
      (reciprocal + Dekker/Markstein correction, bit-exact IEEE divide);
      per t-tile gather-DMA s into row form s_row[1, 1024].
  P2: per token-half th: gpsimd-broadcast s_row half -> s_bc [128,4,512];
      stream xtb tiles (ACT queue) -> DVE mult by s_bc (exact fl(x*s)) ->
      DVE RNE-round via +-1.5*2^23 magic -> bf16 xq_th cache [128,KO,512].
  C:  per (th, nt): stream weight chunks [128,4,512] (SP queue), 4 PSUM banks
      accumulate [128t,512n] over 64 ko (2 (th,nt)-groups in flight); drain
      via ACT Copy(scale=1/s/ws per-token), DVE add bias, store (ACT queue).
      GEMM for th=0 starts while P1/P2 still process th=1.

Exactness: x_q ints in [-127,127] and w {0,1} are exact in bf16; every partial
sum < 2^24 so fp32 PSUM accumulation is exact. round() = +-1.5*2^23 magic (RNE,
matches jnp.round). clip never binds since |x*s| <= 127 by construction.
"""
import sys

sys.path.insert(0, "/opt/trn_rl_repo")

from contextlib import ExitStack

import ml_dtypes
import numpy as np

import concourse.bass as bass
import concourse.tile as tile
from concourse import bacc, mybir
from concourse.bass import ts
from concourse.bass_utils import run_bass_kernel_spmd

TOKENS, IN_F, OUT_F = 4096, 8192, 8192
A_SPLIT, B_SPLIT = 4, 2      # token blocks x outfeature blocks = 8 cores
T_LOC = TOKENS // A_SPLIT    # 1024
N_LOC = OUT_F // B_SPLIT     # 4096
P = 128
KO = IN_F // P               # 64 k-tiles of 128
TT = T_LOC // P              # 8 token tiles
NT = N_LOC // 512            # 8 n-tiles of 512
TH = 2                       # token halves (GEMM pipeline granularity)
TPH = TT // TH               # 4 t-tiles per half
WC = 16                      # weight chunks per n-tile
KO_C = KO // WC              # 4 ko per chunk
XG = 16                      # xt groups per half
XKG = KO // XG               # 4 ko per xt group
MAGIC = float(np.float32(1.5 * 2 ** 23))

_CACHE = {}


def _exact_div127(nc, dst, m, pool, pfx):
    """dst = correctly-rounded IEEE 127/m (f32).

    nc.vector.reciprocal is correctly rounded (verified bit-exact on HW), so
    q0 = fl(127*r0) is within ~1 ulp of 127/m; one Markstein step with an
    exact Dekker residual lands on the correctly-rounded quotient."""
    f32 = mybir.dt.float32
    A = mybir.AluOpType
    sh = list(m.shape)
    t = {k: pool.tile(sh, f32, name=f"{pfx}_{k}", tag=f"dv_{k}")
         for k in ("r0", "q0", "tmp", "hh", "ll", "mh", "ml", "p", "err", "e")}
    nc.vector.reciprocal(t["r0"][:], m[:])
    nc.vector.tensor_scalar_mul(t["q0"][:], t["r0"][:], 127.0)
    C = float(2 ** 12 + 1)
    nc.vector.tensor_scalar_mul(t["tmp"][:], t["q0"][:], C)
    nc.vector.tensor_tensor(t["hh"][:], t["tmp"][:], t["q0"][:], A.subtract)
    nc.vector.tensor_tensor(t["hh"][:], t["tmp"][:], t["hh"][:], A.subtract)
    nc.vector.tensor_tensor(t["ll"][:], t["q0"][:], t["hh"][:], A.subtract)
    nc.vector.tensor_scalar_mul(t["tmp"][:], m[:], C)
    nc.vector.tensor_tensor(t["mh"][:], t["tmp"][:], m[:], A.subtract)
    nc.vector.tensor_tensor(t["mh"][:], t["tmp"][:], t["mh"][:], A.subtract)
    nc.vector.tensor_tensor(t["ml"][:], m[:], t["mh"][:], A.subtract)
    nc.vector.tensor_tensor(t["p"][:], t["q0"][:], m[:], A.mult)
    nc.vector.tensor_tensor(t["err"][:], t["hh"][:], t["mh"][:], A.mult)
    nc.vector.tensor_tensor(t["err"][:], t["err"][:], t["p"][:], A.subtract)
    nc.vector.tensor_tensor(t["tmp"][:], t["hh"][:], t["ml"][:], A.mult)
    nc.vector.tensor_tensor(t["err"][:], t["err"][:], t["tmp"][:], A.add)
    nc.vector.tensor_tensor(t["tmp"][:], t["ll"][:], t["mh"][:], A.mult)
    nc.vector.tensor_tensor(t["err"][:], t["err"][:], t["tmp"][:], A.add)
    nc.vector.tensor_tensor(t["tmp"][:], t["ll"][:], t["ml"][:], A.mult)
    nc.vector.tensor_tensor(t["err"][:], t["err"][:], t["tmp"][:], A.add)
    nc.vector.tensor_scalar(t["e"][:], t["p"][:], 127.0, -1.0, A.subtract, A.mult)
    nc.vector.tensor_tensor(t["e"][:], t["e"][:], t["err"][:], A.subtract)
    nc.vector.tensor_tensor(t["tmp"][:], t["e"][:], t["r0"][:], A.mult)
    nc.vector.tensor_tensor(dst[:], t["q0"][:], t["tmp"][:], A.add)


def _build():
    if "nc" in _CACHE:
        return _CACHE["nc"]

    nc = bacc.Bacc("TRN2", target_bir_lowering=False, debug=False, num_devices=8)
    f32, bf16 = mybir.dt.float32, mybir.dt.bfloat16
    A = mybir.AluOpType

    xb = nc.dram_tensor("xb", [T_LOC, IN_F], f32, kind="ExternalInput").ap()
    xtb = nc.dram_tensor("xtb", [TH, XG, P, XKG, 512], f32,
                         kind="ExternalInput").ap()
    wb = nc.dram_tensor("wb", [NT, WC, P, KO_C, 512], bf16,
                        kind="ExternalInput").ap()
    bb = nc.dram_tensor("bb", [N_LOC], f32, kind="ExternalInput").ap()
    ws = nc.dram_tensor("ws", [1], f32, kind="ExternalInput").ap()
    ob = nc.dram_tensor("ob", [T_LOC, N_LOC], f32, kind="ExternalOutput").ap()

    with tile.TileContext(nc) as tc:
        with ExitStack() as ctx:
            small = ctx.enter_context(tc.tile_pool(name="small", bufs=1))
            xqp = ctx.enter_context(tc.tile_pool(name="xq", bufs=1))
            # per-half k-major x_q caches (64 KB/partition each)
            xq_th = [xqp.tile([P, KO, 512], bf16, name=f"xq{th}")
                     for th in range(TH)]

            # weight-scale reciprocal (per-partition [P,1] broadcast)
            ws_sb = small.tile([1, 1], f32)
            nc.sync.dma_start(ws_sb[:], ws[None, :])
            rws = small.tile([1, 1], f32)
            nc.vector.reciprocal(rws[:], ws_sb[:])
            rws_b = small.tile([P, 1], f32)
            nc.gpsimd.partition_broadcast(rws_b[:], rws[:])

            d_all = small.tile([P, TT], f32)      # per-token out scale 1/s/wscale
            m_all = small.tile([P, TT], f32)
            s_all = small.tile([P, TT], f32)      # s in partition form
            # s in row (free-dim) form, split per token-half so the th=0
            # broadcast doesn't wait on P1 finishing the th=1 tiles.
            # Partition->free transpose must round-trip through DRAM (a
            # partition-crossing SBUF-source DMA silently reads partition 0
            # only); DRAM-space pool tiles get byte-addressed hazard tracking.
            s_rows = [small.tile([1, T_LOC // TH], f32, name=f"srow{th}")
                      for th in range(TH)]
            sdp = ctx.enter_context(tc.tile_pool(name="sdram", bufs=1,
                                                 space="DRAM"))
            s_scrs = [sdp.tile([P, TPH], f32, name=f"sscr{th}")
                      for th in range(TH)]

            # ---- P1: rowmax over x natural -> s (exact), gathered to s_row
            XQRT = 4                     # [128, 2048] quarters
            QW = IN_F // XQRT            # 2048
            with tc.tile_pool(name="p1", bufs=2) as p1p:
                for tt in range(TT):
                    m4 = small.tile([P, XQRT], f32, tag="m4", name=f"m4_{tt}")
                    for q in range(XQRT):
                        xh = p1p.tile([P, QW], f32, tag="xh")
                        nc.sync.dma_start(xh[:], xb[ts(tt, P), ts(q, QW)])
                        nc.vector.tensor_reduce(
                            m4[:, q : q + 1], xh[:], mybir.AxisListType.X,
                            A.max, apply_absolute_value=True)
                    nc.vector.tensor_reduce(m_all[:, tt : tt + 1], m4[:],
                                            mybir.AxisListType.X, A.max)
                    nc.vector.tensor_scalar_max(m_all[:, tt : tt + 1],
                                                m_all[:, tt : tt + 1], 1e-5)
                    s_t = s_all[:, tt : tt + 1]
                    _exact_div127(nc, s_t, m_all[:, tt : tt + 1], small, f"dv{tt}")
                    # d = (1/s) * (1/weight_scale)
                    nc.vector.reciprocal(d_all[:, tt : tt + 1], s_t[:])
                    nc.vector.tensor_scalar(d_all[:, tt : tt + 1],
                                            d_all[:, tt : tt + 1],
                                            rws_b[:, 0:1], None, A.mult)
                    if tt % TPH == TPH - 1:
                        th = tt // TPH
                        # on the SP queue: a blocked entry here only
                        # delays later P1 loads by the DVE tail, while on the
                        # ACT queue it would stall the whole xt prefetch FIFO
                        nc.sync.dma_start(s_scrs[th][:],
                                          s_all[:, ts(th, TPH)])
                        nc.sync.dma_start(
                            s_rows[th][0:1, :],
                            s_scrs[th][:].rearrange("a b -> b a"))

            # ---- P2: quantize transposed x into per-half xq caches ----
            sbp = ctx.enter_context(tc.tile_pool(name="sbc", bufs=1))
            xtp = ctx.enter_context(tc.tile_pool(name="xt", bufs=3))
            for th in range(TH):
                s_bc = sbp.tile([P, XKG, 512], f32, tag="sbc")
                for j in range(XKG):
                    nc.gpsimd.partition_broadcast(
                        s_bc[:, j, :], s_rows[th][0:1, :])
                for g in range(XG):
                    xt = xtp.tile([P, XKG, 512], f32, tag="xt")
                    nc.scalar.dma_start(xt[:], xtb[th, g])
                    # exact fl(x*s): single-rounded IEEE f32 multiply
                    nc.vector.tensor_tensor(xt[:], xt[:], s_bc[:], A.mult)
                    # RNE round to int via (+M, -M), cast bf16 into the cache
                    nc.vector.tensor_scalar(xq_th[th][:, ts(g, XKG), :], xt[:],
                                            MAGIC, -MAGIC, A.add, A.add)

            # ---- C: stream weight chunks, GEMM, drain ----
            slp = ctx.enter_context(tc.tile_pool(name="slab", bufs=4))
            pp = ctx.enter_context(tc.tile_pool(name="psum", bufs=8, space="PSUM"))
            op = ctx.enter_context(tc.tile_pool(name="outp", bufs=2))
            bip = ctx.enter_context(tc.tile_pool(name="bias", bufs=2))

            for th in range(TH):
                for nt in range(NT):
                    b_row = bip.tile([1, 512], f32, tag="brow")
                    nc.scalar.dma_start(b_row[:], bb[None, ts(nt, 512)])
                    b_bc = bip.tile([P, 512], f32, tag="bbc")
                    nc.gpsimd.partition_broadcast(b_bc[:], b_row[:])

                    psums = [pp.tile([P, 512], f32, tag="acc",
                                     name=f"ps_{th}_{nt}_{t}")
                             for t in range(TPH)]
                    for c in range(WC):
                        slab = slp.tile([P, KO_C, 512], bf16, tag="slab")
                        nc.sync.dma_start(slab[:], wb[nt, c])
                        for j in range(KO_C):
                            ko = c * KO_C + j
                            for tl in range(TPH):
                                nc.tensor.matmul(
                                    psums[tl][:],
                                    xq_th[th][:, ko, ts(tl, P)],
                                    slab[:, j, :],
                                    start=(ko == 0), stop=(ko == KO - 1))
                    for tl in range(TPH):
                        t = th * TPH + tl
                        o_sb = op.tile([P, 512], f32, tag="osb")
                        nc.scalar.activation(o_sb[:], psums[tl][:],
                                             mybir.ActivationFunctionType.Copy,
                                             scale=d_all[:, t : t + 1])
                        nc.vector.tensor_tensor(o_sb[:], o_sb[:], b_bc[:], A.add)
                        nc.scalar.dma_start(ob[ts(t, P), ts(nt, 512)], o_sb[:])

    nc.compile()
    _CACHE["nc"] = nc
    return nc


def _pack_weight(weight):
    """Per out-feature shard: [4096, 8192] {0,1} int32 -> bf16 chunk layout
    [NT, WC, P, KO_C, 512], wb[nt,c,kp,j,n'] = w.T[(c*4+j)*128+kp, nt*512+n']."""
    wt = np.ascontiguousarray(weight.T).astype(ml_dtypes.bfloat16)  # [8192, 4096]
    wp = wt.reshape(WC, KO_C, P, NT, 512).transpose(3, 0, 2, 1, 4)
    return np.ascontiguousarray(wp)


def _pack_xt(x_loc):
    """Token block [1024, 8192] f32 -> transposed layout [TH, XG, P, XKG, 512]
    with xtb[th,g,kp,j,t'] = x[th*512+t', (g*4+j)*128+kp]."""
    xt = np.ascontiguousarray(x_loc.T)                 # [8192, 1024]
    xp = xt.reshape(XG, XKG, P, TH, 512).transpose(3, 0, 2, 1, 4)
    return np.ascontiguousarray(xp)


def make_in_maps(x, weight, weight_scale, bias):
    x = np.ascontiguousarray(np.asarray(x, dtype=np.float32))
    weight = np.asarray(weight, dtype=np.int32)
    weight_scale = np.asarray(weight_scale, dtype=np.float32).reshape(1)
    bias = np.ascontiguousarray(np.asarray(bias, dtype=np.float32))

    packs = [_pack_weight(weight[j * N_LOC:(j + 1) * N_LOC])
             for j in range(B_SPLIT)]
    xts = [_pack_xt(x[i * T_LOC:(i + 1) * T_LOC]) for i in range(A_SPLIT)]
    in_maps = []
    for c in range(8):
        i, j = c // B_SPLIT, c % B_SPLIT
        in_maps.append({
            "xb": x[i * T_LOC:(i + 1) * T_LOC],
            "xtb": xts[i],
            "wb": packs[j],
            "bb": bias[j * N_LOC:(j + 1) * N_LOC],
            "ws": weight_scale,
        })
    return in_maps


def kernel(x, weight, weight_scale, bias):
    nc = _build()
    in_maps = make_in_maps(x, weight, weight_scale, bias)
    res = run_bass_kernel_spmd(nc, in_maps, list(range(8))).results

    out = np.empty((TOKENS, OUT_F), dtype=np.float32)
    for c in range(8):
        i, j = c // B_SPLIT, c % B_SPLIT
        out[i * T_LOC:(i + 1) * T_LOC, j * N_LOC:(j + 1) * N_LOC] = res[c]["ob"]
    return out
